# revision 2
# baseline (speedup 1.0000x reference)
"""Distributed Trainium2 Bass kernel for the phasor attention problem
(nn_Attention_17798344475248).

Sharding: 8 cores = 2 batches x 4 head-groups (2 heads each). Per core:
Q/K/V projections for its 2 heads, phasor attention, and a partial final
dense; partials summed with 4 per-quarter fp16 ReduceScatters over the
4-core batch group; atan2 tails run per quarter-pair as RS results land.

v2 changes vs baseline (all fp16 compute — fp8 measured too noisy for the
flip-dominated error budget):
- Both heads processed per chunk; final-dense accumulates h0+h1 in PSUM
  (no SBUF z accumulator), so quarter ReduceScatters fire much earlier
  and the collective payload is fp16 (measured flip-neutral).
- ACT table-set discipline: Abs_reciprocal_sqrt for every normalize
  (probe: 4e-5 max rel err), Sin+Abs for encodes, Exp batched per chunk,
  single-branch Arctan tail -> ~7 table loads total (baseline ~20).
- Normalize chains split across scalar/vector/gpsimd engines.
- bo added after the ReduceScatter from a broadcast tile (no bias matmul).
- Tail: angle = arctan(im/re) + pi*(re<0)*sign(im), one ACT op per tile.
"""
import contextlib
import sys

sys.path.insert(0, "/opt/trn_rl_repo")

import numpy as np

import concourse.bass as bass
import concourse.tile as tile
from concourse import bacc, mybir
from concourse.bass_utils import run_bass_kernel_spmd
from concourse.masks import make_identity

F32 = mybir.dt.float32
F16 = mybir.dt.float16
AF = mybir.ActivationFunctionType
ALU = mybir.AluOpType
PI = float(np.pi)

B, T, D, H = 2, 1024, 512, 8
P = 128
DS = D // P          # 4 partition-slices of the model dim
CH = 512             # kv-side chunk width
NCH = T // CH        # 2 kv chunks
CHQ = 256            # query-side chunk width (1 RS quarter per chunk)
NCHQ = T // CHQ
N_CORES = 8
HPC = 2              # heads per core

Z16 = False          # f32 ReduceScatter payload (fp16 ring-adds cost ~5 flips)


def build(debug=False):
    nc = bacc.Bacc("TRN2", target_bir_lowering=False, debug=False,
                   num_devices=N_CORES)
    cpi2 = nc.alloc_sbuf_tensor("const-f32-pi2", [P, 1], F32)
    nc.gpsimd.memset(cpi2.ap(), PI / 2)
    nc.const_aps.aps[(F32, PI / 2)] = cpi2.ap()
    nc.all_engine_barrier()

    ZDT = F16 if Z16 else F32

    # ---- I/O ----
    QUERY = nc.dram_tensor("query", [T, D], F32, kind="ExternalInput")
    KEYVALUE = nc.dram_tensor("keyvalue", [T, D], F32, kind="ExternalInput")
    WQ = nc.dram_tensor("wq", [HPC, D, D], F32, kind="ExternalInput")
    WK = nc.dram_tensor("wk", [HPC, D, D], F32, kind="ExternalInput")
    WV = nc.dram_tensor("wv", [HPC, D, D], F32, kind="ExternalInput")
    BQ = nc.dram_tensor("bq", [HPC, D], F32, kind="ExternalInput")
    BK = nc.dram_tensor("bk", [HPC, D], F32, kind="ExternalInput")
    BV = nc.dram_tensor("bv", [HPC, D], F32, kind="ExternalInput")
    WO = nc.dram_tensor("wo", [HPC * D, D], F32, kind="ExternalInput")
    BO = nc.dram_tensor("bo", [D], F32, kind="ExternalInput")
    OUT = nc.dram_tensor("out", [T // 4, D], F32, kind="ExternalOutput")

    with tile.TileContext(nc) as tc:
        with contextlib.ExitStack() as ctx:
            pools = {}
            for name, bufs, space in [
                ("persist", 1, "SBUF"),
                ("raw", 2, "SBUF"),       # 2KB x2 raw f32 input tiles
                ("nt", 8, "SBUF"),        # 2KB x8 f32 temps
                ("w", 1, "SBUF"),         # 2KB x1 f32 weight staging
                ("wqk", 4, "SBUF"),       # 4KB x4 fp16 wq/wk
                ("wvo", 4, "SBUF"),       # 4KB x4 fp16 wv/wo
                ("brow", 2, "SBUF"),      # [1,D] bias rows
                ("bcol", 4, "SBUF"),      # [P,DS] bias cols
                ("big16", 4, "SBUF"),     # 8KB x4: kv enc -> q enc -> probs
                ("kt", 4, "SBUF"),        # 8KB x4: K^T fp16 re/im x 2h
                ("v", 4, "SBUF"),         # 8KB x4: V fp16 re/im x 2h
                ("qt", 4, "SBUF"),        # 2KB x4: Q^T fp16 re/im x 2h (per chunk)
                ("qe", 2, "SBUF"),        # 8KB x2: q encodes fp16
                ("oh", 4, "SBUF"),        # 4KB x4: PV out fp16 re/im x 2h
                ("zq", 2, "SBUF"),        # z staging fp16
                ("tail", 2, "SBUF"),      # fp16 rs_out landing tiles
                ("tl", 4, "SBUF"),        # 2KB x4 f32 tail temps
                ("ps", 8, "PSUM"),
                ("dram", 1, "DRAM"),
            ]:
                pools[name] = ctx.enter_context(
                    tc.tile_pool(name=name, bufs=bufs, space=space))

            persist = pools["persist"]
            nt = pools["nt"]
            big16 = pools["big16"]
            ident = persist.tile([P, P], F32, tag="ident")
            make_identity(nc, ident[:])

            # ---- constants / bias prep ----
            ones_f = persist.tile([1, P], F32, tag="onesf")
            nc.vector.memset(ones_f[:], 1.0)
            ones16 = persist.tile([1, P], F16, tag="ones16")
            nc.vector.tensor_copy(ones16[:], ones_f[:])
            bo_row = pools["brow"].tile([1, D], F32, tag="brow", name="bo_row")
            nc.sync.dma_start(bo_row[:], BO[:][None, :])
            bo16 = persist.tile([1, D], F16, tag="bo16")
            nc.vector.tensor_copy(bo16[:], bo_row[:])
            # broadcast bo across 128 partitions (for the post-RS tail add)
            bo_ps = pools["ps"].tile([P, D], F32, tag="ps", name="bo_ps")
            nc.tensor.matmul(bo_ps[:], lhsT=ones16[:], rhs=bo16[:],
                             start=True, stop=True)
            bo128 = persist.tile([P, D], F16, tag="bo128")
            nc.scalar.copy(bo128[:], bo_ps[:])

            bias_cols = {}
            bv16s = {}
            for h in range(HPC):
                bq_col = pools["bcol"].tile([P, DS], F32, tag="bcol",
                                            name=f"bqc{h}")
                nc.sync.dma_start(bq_col[:],
                                  BQ[h].rearrange("(o p) -> p o", p=P))
                bk_col = pools["bcol"].tile([P, DS], F32, tag="bcol",
                                            name=f"bkc{h}")
                nc.sync.dma_start(bk_col[:],
                                  BK[h].rearrange("(o p) -> p o", p=P))
                bv_f = pools["brow"].tile([1, D], F32, tag="brow",
                                          name=f"bvr{h}")
                nc.sync.dma_start(bv_f[:], BV[h][None, :])
                bv16 = persist.tile([1, D], F16, tag=f"bv16_{h}")
                nc.vector.tensor_copy(bv16[:], bv_f[:])
                bias_cols[h] = (bq_col, bk_col)
                bv16s[h] = bv16

            # ---- weights: DMA f32 -> fp16, K/V first (consumed first) ----
            _wq_rr = [0]
            wq_t, wk_t, wv_t, wo_t = {}, {}, {}, {}
            for h in range(HPC):
                wk_t[h] = pools["wqk"].tile([P, DS, D], F16, tag="wqk",
                                            name=f"wk16_{h}")
                wq_t[h] = pools["wqk"].tile([P, DS, D], F16, tag="wqk",
                                            name=f"wq16_{h}")
                wv_t[h] = pools["wvo"].tile([P, DS, D], F16, tag="wvo",
                                            name=f"wv16_{h}")
                wo_t[h] = pools["wvo"].tile([P, DS, D], F16, tag="wvo",
                                            name=f"wo16_{h}")
            for W_ap, wt in [(WK[0], wk_t[0]), (WV[0], wv_t[0]),
                             (WK[1], wk_t[1]), (WV[1], wv_t[1]),
                             (WQ[0], wq_t[0]), (WQ[1], wq_t[1]),
                             (WO[0 * D:1 * D, :], wo_t[0]),
                             (WO[1 * D:2 * D, :], wo_t[1])]:
                for do in range(DS):
                    wf = pools["w"].tile([P, D], F32, tag="wf")
                    eng = (nc.gpsimd, nc.scalar)[_wq_rr[0] % 2]
                    _wq_rr[0] += 1
                    eng.dma_start(wf[:], W_ap[do * P:(do + 1) * P, :])
                    nc.vector.tensor_copy(wt[:, do, :], wf[:])

            # ---- DRAM staging for the collective ----
            dram = pools["dram"]
            zbs = [dram.tile([2 * P * 2, D], ZDT, name=f"zb{q}")
                   for q in range(4)]
            rs_outs = [dram.tile([P, D], ZDT, name=f"rsout{q}")
                       for q in range(4)]

            # ---- encodes: [128, DS, T] transposed layout, fp16 ----
            kv_cos = big16.tile([P, DS, T], F16, tag="e16", name="kv_cos")
            kv_sin = big16.tile([P, DS, T], F16, tag="e16", name="kv_sin")
            q_cos = pools["qe"].tile([P, DS, T], F16, tag="qe", name="q_cos")
            q_sin = pools["qe"].tile([P, DS, T], F16, tag="qe", name="q_sin")

            def encode(src_dram, cos_t, sin_t):
                for ch in range(NCH):
                    chsl = slice(ch * CH, (ch + 1) * CH)
                    raw_tiles = []
                    for ts in range(CH // P):
                        rt = pools["raw"].tile([P, D], F32, tag="raw")
                        nc.sync.dma_start(
                            rt[:],
                            src_dram[ch * CH + ts * P: ch * CH + (ts + 1) * P, :])
                        raw_tiles.append(rt)
                    for ds in range(DS):
                        pt = pools["ps"].tile([P, CH], F32, tag="ps")
                        for ts in range(CH // P):
                            nc.tensor.transpose(
                                pt[:, ts * P:(ts + 1) * P],
                                raw_tiles[ts][:, ds * P:(ds + 1) * P], ident[:])
                        nc.scalar.activation(sin_t[:, ds, chsl], pt[:], AF.Sin,
                                             bias=0.0, scale=PI)
                        s2 = nt.tile([P, CH], F32, tag="nt")
                        nc.scalar.activation(s2[:], pt[:], AF.Sin,
                                             bias=0.0, scale=PI / 2)
                        nc.vector.tensor_tensor(s2[:], s2[:], s2[:], ALU.mult)
                        nc.vector.tensor_scalar(cos_t[:, ds, chsl], s2[:],
                                                -2.0, 1.0, ALU.mult, ALU.add)

            encode(KEYVALUE, kv_cos, kv_sin)
            encode(QUERY, q_cos, q_sin)

            # normalize helper: out_re = (pre [+b]) * n, out_im = pim * n,
            # n = rsqrt((pre+b)^2 + pim^2); work split ACT/DVE/Pool
            def normalize(pre, pim, b_ap, out_re, out_im, fd):
                t1 = nt.tile([P, fd], F32, tag="nt")
                nc.scalar.activation(t1[:, :fd], pre[:], AF.Square,
                                     bias=(0.0 if b_ap is None else b_ap),
                                     scale=1.0)
                t2 = nt.tile([P, fd], F32, tag="nt")
                nc.scalar.activation(t2[:, :fd], pim[:], AF.Square,
                                     bias=0.0, scale=1.0)
                nc.gpsimd.tensor_tensor(t1[:, :fd], t1[:, :fd], t2[:, :fd],
                                        ALU.add)
                n = t2
                nc.scalar.activation(n[:, :fd], t1[:, :fd],
                                     AF.Abs_reciprocal_sqrt, bias=0.0,
                                     scale=1.0)
                if b_ap is None:
                    nc.vector.tensor_tensor(out_re, pre[:], n[:, :fd], ALU.mult)
                else:
                    nc.vector.scalar_tensor_tensor(out_re, pre[:], b_ap,
                                                   n[:, :fd], ALU.add, ALU.mult)
                nc.vector.tensor_tensor(out_im, pim[:], n[:, :fd], ALU.mult)

            # ---- per-head persistent tensors ----
            kt_re, kt_im, v_re, v_im = {}, {}, {}, {}
            for h in range(HPC):
                kt_re[h] = pools["kt"].tile([P, DS, T], F16, tag="kt",
                                            name=f"ktre{h}")
                kt_im[h] = pools["kt"].tile([P, DS, T], F16, tag="kt",
                                            name=f"ktim{h}")
                v_re[h] = pools["v"].tile([P, T // P, D], F16, tag="v",
                                          name=f"vre{h}")
                v_im[h] = pools["v"].tile([P, T // P, D], F16, tag="v",
                                          name=f"vim{h}")

            # ======== K + V projections (both heads) ========
            for h in range(HPC):
                bq_col, bk_col = bias_cols[h]
                for ch in range(NCH):
                    chsl = slice(ch * CH, (ch + 1) * CH)
                    # K projection -> K^T [D', t] with per-partition bias
                    for dso in range(DS):
                        pre = pools["ps"].tile([P, CH], F32, tag="ps")
                        pim = pools["ps"].tile([P, CH], F32, tag="ps")
                        for do in range(DS):
                            nc.tensor.matmul(
                                pre[:],
                                lhsT=wk_t[h][:, do, dso * P:(dso + 1) * P],
                                rhs=kv_cos[:, do, chsl], start=(do == 0),
                                stop=(do == DS - 1))
                            nc.tensor.matmul(
                                pim[:],
                                lhsT=wk_t[h][:, do, dso * P:(dso + 1) * P],
                                rhs=kv_sin[:, do, chsl], start=(do == 0),
                                stop=(do == DS - 1))
                        normalize(pre, pim, bk_col[:, dso:dso + 1],
                                  kt_re[h][:, dso, chsl],
                                  kt_im[h][:, dso, chsl], CH)

                    # V projection -> V [t, D] fp16, bias via K=1 matmul
                    for tb in range(CH // P):
                        tsl = slice(ch * CH + tb * P, ch * CH + (tb + 1) * P)
                        pre = pools["ps"].tile([P, D], F32, tag="ps")
                        pim = pools["ps"].tile([P, D], F32, tag="ps")
                        for do in range(DS):
                            nc.tensor.matmul(
                                pre[:], lhsT=kv_cos[:, do, tsl],
                                rhs=wv_t[h][:, do, :], start=(do == 0),
                                stop=False)
                            nc.tensor.matmul(
                                pim[:], lhsT=kv_sin[:, do, tsl],
                                rhs=wv_t[h][:, do, :], start=(do == 0),
                                stop=(do == DS - 1))
                        nc.tensor.matmul(
                            pre[:], lhsT=ones16[:], rhs=bv16s[h][:],
                            start=False, stop=True)
                        trow = ch * (CH // P) + tb
                        normalize(pre, pim, None,
                                  v_re[h][:, trow, :], v_im[h][:, trow, :], D)

            # ======== attention + dense per chunk (heads interleaved) ========
            for ch in range(NCHQ):
                qsl = slice(ch * CHQ, (ch + 1) * CHQ)
                # Q projections for this chunk (ARS set, merges with PV's)
                qts = {}
                for h in range(HPC):
                    bq_col, bk_col = bias_cols[h]
                    qt_re = pools["qt"].tile([P, DS, CHQ], F16, tag="qt",
                                             name=f"qtre{h}_{ch}")
                    qt_im = pools["qt"].tile([P, DS, CHQ], F16, tag="qt",
                                             name=f"qtim{h}_{ch}")
                    qts[h] = (qt_re, qt_im)
                    for dso in range(DS):
                        pre = pools["ps"].tile([P, CHQ], F32, tag="ps")
                        pim = pools["ps"].tile([P, CHQ], F32, tag="ps")
                        for do in range(DS):
                            nc.tensor.matmul(
                                pre[:],
                                lhsT=wq_t[h][:, do, dso * P:(dso + 1) * P],
                                rhs=q_cos[:, do, qsl], start=(do == 0),
                                stop=(do == DS - 1))
                            nc.tensor.matmul(
                                pim[:],
                                lhsT=wq_t[h][:, do, dso * P:(dso + 1) * P],
                                rhs=q_sin[:, do, qsl], start=(do == 0),
                                stop=(do == DS - 1))
                        normalize(pre, pim, bq_col[:, dso:dso + 1],
                                  qt_re[:, dso, :], qt_im[:, dso, :], CHQ)
                pts = {}
                # scores for both heads (Exp batched -> one table load)
                for h in range(HPC):
                    pt_all = big16.tile([P, T // P, CHQ], F16, tag="e16",
                                        name=f"pt{h}_{ch}")
                    pts[h] = pt_all
                    for to in range(T // P):
                        ps_s = pools["ps"].tile([P, CHQ], F32, tag="ps")
                        qt_re, qt_im = qts[h]
                        for do in range(DS):
                            nc.tensor.matmul(
                                ps_s[:],
                                lhsT=kt_re[h][:, do, to * P:(to + 1) * P],
                                rhs=qt_re[:, do, :], start=(do == 0),
                                stop=False)
                        for do in range(DS):
                            nc.tensor.matmul(
                                ps_s[:],
                                lhsT=kt_im[h][:, do, to * P:(to + 1) * P],
                                rhs=qt_im[:, do, :], start=False,
                                stop=(do == DS - 1))
                        nc.scalar.activation(pt_all[:, to, :], ps_s[:], AF.Exp,
                                             bias=0.0, scale=1.0 / D)

                # PV for both heads -> oh fp16 (ARS batched)
                ohs = {}
                for h in range(HPC):
                    oh_re = pools["oh"].tile([P, DS, CHQ], F16, tag="oh",
                                             name=f"ohre{h}_{ch}")
                    oh_im = pools["oh"].tile([P, DS, CHQ], F16, tag="oh",
                                             name=f"ohim{h}_{ch}")
                    ohs[h] = (oh_re, oh_im)
                    for grp in range(2):
                        ps_tiles = {}
                        for dso in (2 * grp, 2 * grp + 1):
                            ps_tiles[(dso, 0)] = pools["ps"].tile(
                                [P, CHQ], F32, tag="ps",
                                name=f"pv_{h}_{ch}_{dso}_re")
                            ps_tiles[(dso, 1)] = pools["ps"].tile(
                                [P, CHQ], F32, tag="ps",
                                name=f"pv_{h}_{ch}_{dso}_im")
                        for to in range(T // P):
                            for dso in (2 * grp, 2 * grp + 1):
                                nc.tensor.matmul(
                                    ps_tiles[(dso, 0)][:],
                                    lhsT=v_re[h][:, to, dso * P:(dso + 1) * P],
                                    rhs=pts[h][:, to, :], start=(to == 0),
                                    stop=(to == T // P - 1))
                                nc.tensor.matmul(
                                    ps_tiles[(dso, 1)][:],
                                    lhsT=v_im[h][:, to, dso * P:(dso + 1) * P],
                                    rhs=pts[h][:, to, :], start=(to == 0),
                                    stop=(to == T // P - 1))
                        for dso in (2 * grp, 2 * grp + 1):
                            normalize(ps_tiles[(dso, 0)], ps_tiles[(dso, 1)],
                                      None, oh_re[:, dso, :], oh_im[:, dso, :],
                                      CHQ)

                # dense: accumulate BOTH heads per ts tile, stage z, fire RS
                for ts in range(CHQ // P):
                    pzre = pools["ps"].tile([P, D], F32, tag="ps")
                    pzim = pools["ps"].tile([P, D], F32, tag="ps")
                    for h in range(HPC):
                        oh_re, oh_im = ohs[h]
                        for do in range(DS):
                            nc.tensor.matmul(
                                pzre[:],
                                lhsT=oh_re[:, do, ts * P:(ts + 1) * P],
                                rhs=wo_t[h][:, do, :],
                                start=(h == 0 and do == 0),
                                stop=(h == HPC - 1 and do == DS - 1))
                            nc.tensor.matmul(
                                pzim[:],
                                lhsT=oh_im[:, do, ts * P:(ts + 1) * P],
                                rhs=wo_t[h][:, do, :],
                                start=(h == 0 and do == 0),
                                stop=(h == HPC - 1 and do == DS - 1))
                    zre16 = pools["zq"].tile([P, D], ZDT, tag="zq")
                    zim16 = pools["zq"].tile([P, D], ZDT, tag="zq")
                    nc.scalar.copy(zre16[:], pzre[:])
                    nc.vector.tensor_copy(zim16[:], pzim[:])
                    tq0 = ch * CHQ + ts * P
                    qq = tq0 // 256
                    r0 = 2 * ((tq0 % 256) // P)  # 0 or 2
                    for half in range(2):
                        r_ = r0 + half
                        src = slice(half * 64, (half + 1) * 64)
                        nc.sync.dma_start(
                            zbs[qq][r_ * P: r_ * P + 64, :], zre16[src, :])
                        nc.scalar.dma_start(
                            zbs[qq][r_ * P + 64: r_ * P + 128, :],
                            zim16[src, :])
                    if ts % 2 == 1:
                        nc.gpsimd.collective_compute(
                            "ReduceScatter", ALU.add,
                            replica_groups=[[0, 1, 2, 3], [4, 5, 6, 7]],
                            ins=[zbs[qq].opt()],
                            outs=[rs_outs[qq].opt()],
                        )

            # ======== tail: angle = arctan(im/re) + pi*(re<0)*sgn(im) ========
            # out/pi = arctan(im/re)/pi + (re<0 ? sgn(im) : 0)
            # tile_wait_until: keep tail ops at the BACK of every engine
            # queue -- the scheduler does not model collective latency and
            # otherwise hoists these RS-dependent ops ahead of live compute
            # (measured 41us全-engine stall).
            for pp in range(2):
                tc.tile_set_cur_wait(0.6 + 0.1 * pp)
                qa, qb = 2 * pp, 2 * pp + 1
                lre = pools["tail"].tile([P, D], ZDT, tag="tail",
                                         name=f"lre{pp}")
                lim = pools["tail"].tile([P, D], ZDT, tag="tail",
                                         name=f"lim{pp}")
                nc.sync.dma_start(lre[0:64, :], rs_outs[qa][0:64, :])
                nc.sync.dma_start(lim[0:64, :], rs_outs[qa][64:128, :])
                nc.sync.dma_start(lre[64:128, :], rs_outs[qb][0:64, :])
                nc.sync.dma_start(lim[64:128, :], rs_outs[qb][64:128, :])

                def ft(nm, pp=pp):
                    return pools["tl"].tile([P, D], F32, tag="tl",
                                            name=f"{nm}{pp}")
                # liveness-ordered allocations (tag rotates 4 bufs)
                zre_t, a, at, mask = ft("zre"), ft("a"), ft("at"), ft("ms")
                nc.vector.tensor_tensor(zre_t[:], lre[:], bo128[:], ALU.add)
                for hw in range(2):
                    cs = slice(hw * (D // 2), (hw + 1) * (D // 2))
                    zre = zre_t[:, cs]
                    zim = lim[:, cs]
                    nc.vector.reciprocal_approx_fast(a[:, cs], zre)
                    nc.vector.tensor_tensor(a[:, cs], zim, a[:, cs], ALU.mult)
                    nc.vector.tensor_scalar(a[:, cs], a[:, cs], 1e8, -1e8,
                                            ALU.min, ALU.max)
                    nc.scalar.activation(at[:, cs], a[:, cs], AF.Arctan,
                                         bias=0.0, scale=1.0)
                    # mask = (re<0); sgn(in a) = 2*(im>=0)-1; w = mask*sgn
                    nc.vector.tensor_scalar(mask[:, cs], zre, 0.0, None,
                                            ALU.is_lt)
                    nc.vector.tensor_scalar(a[:, cs], zim, 0.0, None,
                                            ALU.is_ge)
                    nc.vector.tensor_scalar(a[:, cs], a[:, cs], 2.0, -1.0,
                                            ALU.mult, ALU.add)
                    nc.vector.tensor_tensor(mask[:, cs], mask[:, cs], a[:, cs],
                                            ALU.mult)
                    nc.vector.scalar_tensor_tensor(at[:, cs], at[:, cs],
                                                   1.0 / PI, mask[:, cs],
                                                   ALU.mult, ALU.add)
                nc.sync.dma_start(OUT[pp * P:(pp + 1) * P, :], at[:, :])

    nc.finalize()
    return nc


_NC_CACHE = {}


def _get_nc():
    if "nc" not in _NC_CACHE:
        _NC_CACHE["nc"] = build()
    return _NC_CACHE["nc"]


def kernel(**inputs):
    query = np.ascontiguousarray(np.asarray(inputs["query"], dtype=np.float32))
    keyvalue = np.ascontiguousarray(
        np.asarray(inputs["keyvalue"], dtype=np.float32))
    wq = np.asarray(inputs["wq"], dtype=np.float32)
    wk = np.asarray(inputs["wk"], dtype=np.float32)
    wv = np.asarray(inputs["wv"], dtype=np.float32)
    bq = np.asarray(inputs["bq"], dtype=np.float32)
    bk = np.asarray(inputs["bk"], dtype=np.float32)
    bv = np.asarray(inputs["bv"], dtype=np.float32)
    wo = np.asarray(inputs["wo"], dtype=np.float32)
    bo = np.asarray(inputs["bo"], dtype=np.float32)

    in_maps = []
    for c in range(N_CORES):
        b, g = c // 4, c % 4
        h0 = g * HPC
        in_maps.append({
            "query": query[b],
            "keyvalue": keyvalue[b],
            "wq": np.ascontiguousarray(wq[h0:h0 + HPC]),
            "wk": np.ascontiguousarray(wk[h0:h0 + HPC]),
            "wv": np.ascontiguousarray(wv[h0:h0 + HPC]),
            "bq": np.ascontiguousarray(bq[h0:h0 + HPC]),
            "bk": np.ascontiguousarray(bk[h0:h0 + HPC]),
            "bv": np.ascontiguousarray(bv[h0:h0 + HPC]),
            "wo": np.ascontiguousarray(wo[h0 * D:(h0 + HPC) * D]),
            "bo": bo,
        })

    nc = _get_nc()
    res = run_bass_kernel_spmd(nc, in_maps, core_ids=list(range(N_CORES)))
    _NC_CACHE["last_results"] = res
    out = np.empty((B, T, D), np.float32)
    for c in range(N_CORES):
        b, g = c // 4, c % 4
        o = res.results[c]["out"]          # [256, 512]: 4 quarters x 64 rows
        for qq in range(4):
            out[b, qq * 256 + g * 64: qq * 256 + (g + 1) * 64, :] = \
                o[qq * 64:(qq + 1) * 64, :]
    return out
